# revision 37
# baseline (speedup 1.0000x reference)
"""Causal self-attention (B=4, T=2048, D=1024, H=16) on 8 TRN2 NeuronCores.

Sharding: core c -> (batch b = c//2, head-group g = c%2 of 8 heads).
Each core computes QKV projection for its 8 heads, causal attention, and a
partial out-projection (its heads' rows of W_out). The two partials per batch
are summed on the host during unshard (the "all-reduce after out_proj" of the
tensor-parallel scheme).

Projections (QKV, out-proj) run as error-compensated fp8 DoubleRow matmuls
(2 contraction chunks per instruction at 0.5 cycles/row): for a = a_hi + a_lo
(both fp8e4), a@b = a_hi@b_hi + a_lo@b_hi + a_hi@b_lo, dropping the lo*lo
term (~1e-3 relative, below bf16 noise; end-to-end rel err 6.5e-3 vs the
fp32 reference, matching an all-bf16 kernel). Weights are pre-scaled by 32 on
the host so W' ~ N(0,1) splits cleanly in e4m3; the scale folds into the exp
argument (2^-13), the softmax-denominator ones column (32.0), and the final
1/32 output copy. x / W_qkv / W_out are pre-split (x also pre-transposed)
on the host and DMAd as fp8 bytes straight into their SBUF layouts -- no
on-device cast loads, PE transposes, or staging. attn^T is split into fp8
hi/lo right after each DMA-transpose chunk lands (Pool mid-kernel, DVE at
the tail). Output is written as fp16; partial-sum + b_out run on the host.

Attention core (bf16): scoresT [k,q] per head-pair in PSUM (2 heads packed
into partition halves of the PE), exp on ScalarE (fp32->bf16, live query
sub-ranges only), 0/1-mask multiply on the four diagonal 128x128 blocks
(DVE), AV in the flipped orientation out[q,65] = P^T @ V_aug with a
ones-column denominator, packed 4-chains-per-PSUM-bank; VectorE reciprocal +
broadcast multiplies produce attn [t, dloc] bf16. Dead (k>q) subtiles are
skipped entirely.

Scheduling: qt-outer passes (pass qt runs units hp=0..3). Work that is
exp-independent -- next-pass qk/vproj chains and completed rows' out-proj
chains -- is held in keyed queues and drained as fillers inside the
attention chunks exactly where the PE would stall on exp latency (passes 2/3
are exp-bound, so out-proj supply is concentrated there); keys force-drain
anything a unit reads before it is emitted. All input DMAs ride one SP-queue
stream in exact consumption order (DMA-completion sems count per queue, so
a consumer of the k-th DMA waits for all earlier ones anyway), keeping the
ACT queue free of DMA issues that would delay exp dispatch. The last unit
finishes its two accumulator banks separately so normalize / transpose /
fp8-split / out-proj of the first half overlap the second half's AV matmuls,
with the final out DMAs issued round-robin across queues.
"""

import numpy as np
import ml_dtypes

import concourse.bass as bass
import concourse.mybir as mybir
import concourse.tile as tile
from concourse.bass_utils import run_bass_kernel_spmd

F32 = mybir.dt.float32
BF16 = mybir.dt.bfloat16
FP16 = mybir.dt.float16
FP8 = mybir.dt.float8e4
AX = mybir.AluOpType
DR = mybir.MatmulPerfMode.DoubleRow

T = 2048
D = 1024
HLOC = 8          # heads per core
DKH = 64
QT = 512          # query tile
NQT = T // QT     # 4
NDIN = D // 128   # 8
NMT = 4           # q/k feature m-tiles (512 local feats / 128)
VA = 65           # V cols per head incl. ones column
NTT = T // 128    # 16
EXP = mybir.ActivationFunctionType.Exp
WSCALE = 32.0     # host-side weight scale (folded back out on device)
EXPSCALE = 0.125 / (WSCALE * WSCALE)   # 2^-13


_NOP_ID = [0]


def _split_multiwaits(nc, limit=1):
    """This toolchain's walrus rejects more than one sync-wait on an
    instruction ("Too many sync wait commands"), and rejects any sync-wait
    on the DMA-xpose descriptor. Move excess waits onto same-engine NOPs
    inserted immediately before the instruction — the engine sequencer
    executes them in program order, so semantics are preserved."""
    for f in nc.m.functions:
        for blk in f.blocks:
            new = []
            changed = False
            for inst in blk.instructions:
                si = inst.sync_info
                lim = 0 if type(inst).__name__ == "InstDmaTransposeAnt" else limit
                if si is not None and len(si.on_wait) > lim:
                    waits = list(si.on_wait)
                    inst.sync_info = mybir.SyncInfo(
                        on_wait=waits[:lim], on_update=list(si.on_update))
                    for w in waits[lim:]:
                        _NOP_ID[0] += 1
                        nop = mybir.InstNoOp(
                            name=f"waitnop-{_NOP_ID[0]}", ins=[], outs=[])
                        nop.engine = inst.engine
                        nop.sync_info = mybir.SyncInfo(on_wait=[w], on_update=[])
                        new.append(nop)
                    changed = True
                new.append(inst)
            if changed:
                blk.instructions = new


def build_nc():
    nc = bass.Bass()
    # host-prepped fp8 operands, already in SBUF layout (see make_in_maps)
    x8_ext = nc.declare_dram_parameter("x8", [128, 2 * NQT * NDIN * 512], FP8, isOutput=False)
    wqk_ext = nc.declare_dram_parameter("wqk8", [128, 2 * NMT * 2 * NDIN * 128], FP8, isOutput=False)
    wv_ext = nc.declare_dram_parameter("wv8", [128, 2 * NDIN * 512], FP8, isOutput=False)
    wo_ext = nc.declare_dram_parameter("wo8", [128, 2 * 4 * D], FP8, isOutput=False)
    b_ext = nc.declare_dram_parameter("b_qkv", [3 * 512], F32, isOutput=False)
    out_ext = nc.declare_dram_parameter("out", [T, D], FP16, isOutput=True)

    with tile.TileContext(nc) as tc:
        with (
            tc.tile_pool(name="const", bufs=1) as constp,
            tc.tile_pool(name="big", bufs=1) as bigp,
        ):
            # causal 0/1 mask [128, 2x512] bf16 (head-duplicated triangle):
            # mask[p, (h, f)] = 1 if f >= p else 0. Diagonal 128x128 blocks
            # use the [:, :, 0:128] slice.
            maskt = constp.tile([128, 1024], BF16, tag="maskt")

            def emit_mask_gen():
                nc.gpsimd.memset(maskt, 1.0)
                mk3 = maskt.rearrange("p (h f) -> p h f", f=512)
                nc.gpsimd.affine_select(
                    out=mk3, in_=mk3,
                    compare_op=AX.is_ge, fill=0.0,
                    base=0, channel_multiplier=-1,
                    pattern=[[0, 2], [1, 512]],
                )

            # biases (already x32 on host): per-partition vectors for q/k
            # feature tiles, broadcast tile for V (bias along the free axis)
            bq_sb = constp.tile([128, NMT], F32, tag="bq")
            bk_sb = constp.tile([128, NMT], F32, tag="bk")
            bv_sb = constp.tile([128, 512], F32, tag="bv")

            def emit_bias_loads():
                nc.scalar.dma_start(
                    out=bq_sb,
                    in_=b_ext[0:512].rearrange("(m p) -> p m", p=128))
                nc.scalar.dma_start(
                    out=bk_sb,
                    in_=b_ext[512:1024].rearrange("(m p) -> p m", p=128))
                bv_src = b_ext[1024:1536]
                nc.scalar.dma_start(
                    out=bv_sb,
                    in_=bass.AP(tensor=bv_src.tensor, offset=bv_src.offset,
                                ap=[[0, 128]] + list(bv_src.ap)),
                )

            # persistent tiles
            qT = bigp.tile([128, NMT * T], BF16, tag="qT")
            kT = bigp.tile([128, NMT * T], BF16, tag="kT")
            vaug = bigp.tile([128, NTT * HLOC * VA], BF16, tag="vaug")
            attn = bigp.tile([128, NTT * 512], BF16, tag="attn")
            attnT = bigp.tile([128, 4 * T], BF16, tag="attnT")
            a8h = bigp.tile([128, 4 * T], FP8, tag="a8h")
            a8l = bigp.tile([128, 4 * T], FP8, tag="a8l")
            from types import SimpleNamespace as _NS

            def _sub(t, off):
                return _NS(tensor=t.tensor, offset=t.offset + off, ap=t.ap)

            x8 = bigp.tile([128, 2 * NDIN * T], FP8, tag="x8")
            wqk8 = bigp.tile([128, 2 * NMT * 2 * NDIN * 128], FP8, tag="wqk8")
            wv8 = bigp.tile([128, 2 * NDIN * 512], FP8, tag="wv8")
            wo8 = bigp.tile([128, 2 * 4 * D], FP8, tag="wo8")
            xT8h, xT8l = _sub(x8, 0), _sub(x8, NDIN * T)
            wqk8h, wqk8l = _sub(wqk8, 0), _sub(wqk8, 8192)
            wv8h, wv8l = _sub(wv8, 0), _sub(wv8, NDIN * 512)
            wo8h, wo8l = _sub(wo8, 0), _sub(wo8, 4 * D)

            with (
                tc.tile_pool(name="pjpsum", bufs=2, space="PSUM") as pjpsum,
            ):
                # ---- loads: plain fp8 byte DMAs, ordered so the first qk
                # chains' operands land first. x t-group g is contiguous in
                # the host layout; the SBUF dst is kk-strided.
                def load_xg(g, q):
                    # one DMA per t-group covering hi and lo halves
                    q.dma_start(
                        out=bass.AP(
                            tensor=x8.tensor, offset=x8.offset + g * 512,
                            ap=[list(x8.ap[0]), [NDIN * T, 2], [T, NDIN],
                                [1, 512]]),
                        in_=x8_ext[:, g * 8192:(g + 1) * 8192],
                    )

                def load_wqk_mt(mt, q):
                    q.dma_start(
                        out=bass.AP(
                            tensor=wqk8.tensor,
                            offset=wqk8.offset + mt * 2048,
                            ap=[list(wqk8.ap[0]), [8192, 2], [1, 2048]]),
                        in_=wqk_ext[:, mt * 4096:(mt + 1) * 4096],
                    )

                # one linear load stream on SP in exact consumption order:
                # DMA-completion sems count per queue, so a consumer of the
                # k-th DMA waits for all earlier ones on the path anyway
                emit_mask_gen()
                load_wqk_mt(0, nc.sync)
                load_xg(0, nc.sync)
                emit_bias_loads()
                nc.sync.dma_start(out=wv8, in_=wv_ext[:, :])
                load_wqk_mt(1, nc.sync)
                load_xg(1, nc.sync)
                load_xg(2, nc.sync)
                load_wqk_mt(2, nc.sync)
                load_wqk_mt(3, nc.sync)
                nc.sync.dma_start(out=wo8, in_=wo_ext[:, :])
                load_xg(3, nc.sync)

                def dr_mm(ps, wt, xt, woff, wstride, xoff, xstride, nmov,
                          start, stop, tag=""):
                    nc.tensor.matmul(
                        ps,
                        lhsT=bass.AP(
                            tensor=wt.tensor, offset=wt.offset + woff,
                            ap=[list(wt.ap[0]), [wstride, 2], [1, 128]]),
                        rhs=bass.AP(
                            tensor=xt.tensor, offset=xt.offset + xoff,
                            ap=[list(xt.ap[0]), [xstride, 2], [1, nmov]]),
                        start=start, stop=stop, perf_mode=DR,
                    ).annotate(tag)

                def qk_chain(mt, sec, n):
                    dst, bias = (qT, bq_sb) if sec == 0 else (kT, bk_sb)
                    ps = pjpsum.tile([128, 512], F32, tag="pj")
                    nmm = 0
                    for wt, xt in ((wqk8h, xT8h), (wqk8l, xT8h), (wqk8h, xT8l)):
                        for p in range(4):
                            dr_mm(ps, wt, xt,
                                  mt * 2048 + sec * 1024 + (2 * p) * 128, 128,
                                  (2 * p) * T + n * 512, T, 512,
                                  start=(nmm == 0), stop=(nmm == 11),
                                  tag=f"qk:{mt}.{sec}.{n}")
                            nmm += 1
                    nc.vector.tensor_scalar(
                        out=dst[:, mt * T + n * 512: mt * T + (n + 1) * 512],
                        in0=ps, scalar1=bias[:, mt:mt + 1], scalar2=None,
                        op0=AX.add)

                def emit_qkproj(mt, ns=None):
                    for sec in (0, 1):
                        for n in (range(NQT) if ns is None else ns):
                            qk_chain(mt, sec, n)

                def emit_vproj(tts):
                    for tt in tts:
                        ps = pjpsum.tile([128, 512], F32, tag="pj")
                        nmm = 0
                        for xt, wt in ((xT8h, wv8h), (xT8l, wv8h), (xT8h, wv8l)):
                            for p in range(4):
                                dr_mm(ps, xt, wt,
                                      (2 * p) * T + tt * 128, T,
                                      (2 * p) * 512, 512, 512,
                                      start=(nmm == 0), stop=(nmm == 11),
                                      tag=f"vp:{tt}")
                                nmm += 1
                        blk = vaug[:, tt * (HLOC * VA):(tt + 1) * (HLOC * VA)]
                        blk3 = blk.rearrange("p (h c) -> p h c", c=VA)
                        nc.vector.tensor_tensor(
                            out=blk3[:, :, 0:64],
                            in0=ps.rearrange("p (h c) -> p h c", c=64),
                            in1=bv_sb.rearrange("p (h c) -> p h c", c=64),
                            op=AX.add)
                        nc.vector.memset(blk3[:, :, 64:65], WSCALE)

                with (
                    tc.tile_pool(name="scps", bufs=2, space="PSUM") as scps,
                    tc.tile_pool(name="avps", bufs=1, space="PSUM") as avps,
                    tc.tile_pool(name="ptp", bufs=16) as ptp,
                    tc.tile_pool(name="recp", bufs=4) as recp,
                    tc.tile_pool(name="yo", bufs=10) as yo,
                ):
                    def av_off(j, par):
                        s = 2 * j + par
                        return (s // 4) * 512 + (s % 4) * 65

                    def emit_unit(hp, qt, fillers=None, per_slot=1,
                                  tail=False):
                        def fill(k):
                            for _ in range(k):
                                if fillers:
                                    fillers.popleft()()

                        if fillers is not None:
                            fillers.reserve = 0 if tail else 4

                        def av_mm(kt, j, par, pt_t):
                            h = 2 * hp + par
                            off = av_off(j, par)
                            # PSUM start/stop are per 2KB bank (zero
                            # region): exactly one start (zeroes the bank)
                            # and one stop per bank; sibling chains ride.
                            nc.tensor.matmul(
                                avt[:, off:off + VA],
                                lhsT=pt_t[:, par * 512 + j * 128:
                                          par * 512 + (j + 1) * 128],
                                rhs=vaug[:, kt * (HLOC * VA) + h * VA:
                                         kt * (HLOC * VA) + (h + 1) * VA],
                                start=(kt == 0 and par == 0 and j % 2 == 0),
                                stop=(par == 1 and j % 2 == 1
                                      and kt == 4 * qt + j),
                            ).annotate(f"av:{hp}.{qt}.{kt}.{j}.{par}")

                        nkt = 4 * (qt + 1)
                        avt = avps.tile([128, 1024], F32, tag="av")
                        for c0 in range(0, nkt, 8):
                            is_last = tail and c0 + 8 >= nkt
                            chunk = list(range(c0, min(c0 + 8, nkt)))
                            pts = {}
                            for kt in chunk:
                                # diagonal k-tile (i >= 0): only queries
                                # f >= 128*i are live
                                i = kt - (nkt - 4)
                                lo_q = max(0, 128 * i)
                                ps = scps.tile([128, 1024], F32, tag="sc")
                                for par in range(2):
                                    lo, hi = par * 64, par * 64 + 64
                                    nc.tensor.matmul(
                                        ps[:, par * 512 + lo_q:(par + 1) * 512],
                                        lhsT=kT[lo:hi, hp * T + kt * 128:
                                                hp * T + (kt + 1) * 128],
                                        rhs=qT[lo:hi, hp * T + qt * 512 + lo_q:
                                               hp * T + (qt + 1) * 512],
                                        start=True, stop=True,
                                    ).annotate(f"sc:{hp}.{qt}.{kt}")
                                pt_t = ptp.tile([128, 1024], BF16, tag="pt")
                                ps3 = ps.rearrange("p (h q) -> p h q", q=512)
                                pt3 = pt_t.rearrange("p (h q) -> p h q", q=512)
                                nc.scalar.activation(
                                    pt3[:, :, lo_q:512], ps3[:, :, lo_q:512],
                                    EXP, bias=0.0, scale=EXPSCALE)
                                if i >= 0:
                                    # mask the diagonal 128x128 block (both
                                    # heads): q-subtile j == i
                                    blk = bass.AP(
                                        tensor=pt_t.tensor,
                                        offset=pt_t.offset + lo_q,
                                        ap=[list(pt_t.ap[0]), [512, 2], [1, 128]])
                                    mblk = bass.AP(
                                        tensor=maskt.tensor,
                                        offset=maskt.offset,
                                        ap=[list(maskt.ap[0]), [512, 2], [1, 128]])
                                    nc.vector.tensor_tensor(
                                        out=blk, in0=blk, in1=mblk, op=AX.mult)
                                pts[kt] = pt_t
                                if qt == 3 and (kt - c0) in (3, 5, 7):
                                    fill(1)
                            fill(1 if is_last else per_slot)
                            drain_due(qt, hp, 1)
                            if not is_last:
                                for kt in chunk:
                                    for j in range(4):
                                        if kt > 4 * qt + j:
                                            continue  # fully-dead block
                                        for par in range(2):
                                            av_mm(kt, j, par, pts[kt])
                            else:
                                # tail: finish bank 0 (q-subtiles 0,1) first
                                # so its normalize / attnT transpose /
                                # out-proj overlap bank 1's AV matmuls; the
                                # final out-proj chains start per-bank as
                                # soon as their fp8 splits land
                                for bank in range(2):
                                    for kt in chunk:
                                        for j in (2 * bank, 2 * bank + 1):
                                            if kt > 4 * qt + j:
                                                continue
                                            for par in range(2):
                                                av_mm(kt, j, par, pts[kt])
                                    if bank == 0:
                                        normalize(hp, qt, avt, bank=0)
                                        xpose_tt(4 * qt)
                                        xpose_tt(4 * qt + 1)
                                        split_tt(4 * qt, on_dve=True)
                                        split_tt(4 * qt + 1)
                                        fill(2)
                                    else:
                                        emit_outproj([4 * qt, 4 * qt + 1])
                                        normalize(hp, qt, avt, bank=1)
                                        xpose_tt(4 * qt + 2)
                                        xpose_tt(4 * qt + 3)
                                        split_tt(4 * qt + 2, on_dve=True)
                                        split_tt(4 * qt + 3)
                                        fill(len(fillers))
                                        rot = [nc.sync, nc.scalar,
                                               nc.gpsimd, nc.scalar]
                                        for i, (m, nn) in enumerate(
                                                (m, nn)
                                                for m in (4 * qt + 2, 4 * qt + 3)
                                                for nn in range(2)):
                                            out_chain(m, nn, qq=rot[i])
                        if not tail:
                            normalize(hp, qt, avt)

                    def normalize(hp, qt, avt, bank=None):
                        # rc[p, s] = 1 / denom(slot s); attn chunk cols
                        # (hp, par) of t-chunks 4qt..4qt+3 (or one bank's 2)
                        if bank is None:
                            rc = recp.tile([128, 8], F32, tag="rc")
                            nc.vector.reciprocal(
                                rc,
                                bass.AP(tensor=avt.tensor,
                                        offset=avt.offset + 64,
                                        ap=[list(avt.ap[0]), [512, 2],
                                            [130, 2], [65, 2]]))
                        else:
                            rc = recp.tile([128, 4], F32, tag="rc")
                            nc.vector.reciprocal(
                                rc,
                                bass.AP(tensor=avt.tensor,
                                        offset=avt.offset + bank * 512 + 64,
                                        ap=[list(avt.ap[0]), [130, 2],
                                            [65, 2]]))
                        base = 0 if bank is None else bank * 512
                        j0 = 0 if bank is None else 2 * bank
                        for par in range(2):
                            if bank is None:
                                dstd = [[1024, 2], [512, 2]]
                                srcd = [[512, 2], [130, 2]]
                                rcd = [[4, 2], [2, 2]]
                            else:
                                dstd = [[512, 2]]
                                srcd = [[130, 2]]
                                rcd = [[2, 2]]
                            dst = bass.AP(
                                tensor=attn.tensor,
                                offset=attn.offset + (4 * qt + j0) * 512
                                + (hp * 2 + par) * 64,
                                ap=[list(attn.ap[0])] + dstd + [[1, 64]])
                            src = bass.AP(
                                tensor=avt.tensor,
                                offset=avt.offset + base + par * 65,
                                ap=[list(avt.ap[0])] + srcd + [[1, 64]])
                            rcb = bass.AP(
                                tensor=rc.tensor, offset=rc.offset + par,
                                ap=[list(rc.ap[0])] + rcd + [[0, 64]])
                            nc.vector.tensor_tensor(
                                out=dst, in0=src, in1=rcb, op=AX.mult)

                    def xpose_tt(tt):
                        nc.sync.dma_start_transpose(
                            out=bass.AP(
                                tensor=attnT.tensor,
                                offset=attnT.offset + tt * 128,
                                ap=[list(attnT.ap[0]), [T, 4], [1, 128]]),
                            in_=attn[:, tt * 512:(tt + 1) * 512])

                    def split_tt(tt, on_dve=False):
                        # fp8 hi/lo split of attnT columns tt*128..+128 of
                        # all 4 dloc chunks. Runs on the (otherwise idle)
                        # Pool engine so the waits on the DMA transposes
                        # don't head-of-line-block the DVE queue, which
                        # carries the normalize/bias ops the PE depends on.
                        sel = [[T, 4], [1, 128]]
                        dsth = bass.AP(tensor=a8h.tensor,
                                       offset=a8h.offset + tt * 128,
                                       ap=[list(a8h.ap[0])] + sel)
                        dstl = bass.AP(tensor=a8l.tensor,
                                       offset=a8l.offset + tt * 128,
                                       ap=[list(a8l.ap[0])] + sel)
                        src = bass.AP(tensor=attnT.tensor,
                                      offset=attnT.offset + tt * 128,
                                      ap=[list(attnT.ap[0])] + sel)
                        eng = nc.vector if on_dve else nc.gpsimd
                        eng.tensor_copy(dsth, src)
                        eng.tensor_tensor(
                            out=dstl, in0=src, in1=dsth, op=AX.subtract)

                    def emit_attnT(qt):
                        for j in range(4):
                            xpose_tt(4 * qt + j)
                            split_tt(4 * qt + j)

                    def out_chain(mt, n, alt=False, qq=None):
                        ps = pjpsum.tile([128, 512], F32, tag="pj", name="y")
                        nmm = 0
                        for at, wt in ((a8h, wo8h), (a8l, wo8h), (a8h, wo8l)):
                            for p in range(2):
                                dr_mm(ps, at, wt,
                                      (2 * p) * T + mt * 128, T,
                                      (2 * p) * D + n * 512, D, 512,
                                      start=(nmm == 0), stop=(nmm == 5),
                                      tag=f"out:{mt}.{n}")
                                nmm += 1
                        yt = yo.tile([128, 512], FP16, tag="yt", name="yt")
                        nc.vector.tensor_scalar(
                            out=yt, in0=ps, scalar1=1.0 / WSCALE, scalar2=None,
                            op0=AX.mult)
                        q = qq if qq is not None else (
                            nc.scalar if alt else nc.sync)
                        q.dma_start(
                            out=out_ext[mt * 128:(mt + 1) * 128,
                                        n * 512:(n + 1) * 512], in_=yt)

                    def emit_outproj(mts, alt=False):
                        for mt in mts:
                            for n in range(2):
                                out_chain(mt, n, alt=alt)

                    from collections import deque

                    # qt-outer pass structure: pass qt runs units (hp=0..3,
                    # qt). Projection chains for the NEXT pass (qk n=qt+1,
                    # vproj of the next four t-tiles) and out-proj chains of
                    # the PREVIOUS pass's completed attn rows are drained as
                    # fillers between an attention chunk's exp and its AV
                    # matmuls, so the PE has exp-independent work exactly
                    # where it would stall. projq (projection work, required
                    # by the next pass) drains before outq (out-proj, only
                    # required at the end) and is force-drained at pass end.
                    # projq entries are (key, fn) with key = (qt, hp) of the
                    # FIRST unit that reads the result; entries are pushed in
                    # key order and force-drained before emitting that unit,
                    # so emission order always respects dataflow even when
                    # the fill slots didn't get to them. outq (out-proj) has
                    # no downstream consumer until the end.
                    projq = deque()
                    outq = deque()

                    class FillQ:
                        reserve = 0

                        def popleft(self):
                            if projq:
                                return projq.popleft()[1]
                            if len(outq) > self.reserve:
                                return outq.popleft()
                            return lambda: None

                        def __bool__(self):
                            return bool(projq) or bool(outq)

                        def __len__(self):
                            return len(projq) + len(outq)

                    fillers = FillQ()

                    def drain_due(qt, hp, phase=0):
                        while projq and projq[0][0] <= (qt, hp, phase):
                            projq.popleft()[1]()

                    def push_qk(n, mts=None):
                        for mt in (range(NMT) if mts is None else mts):
                            for sec in (0, 1):
                                projq.append(((n, mt, 0),
                                              (lambda m, s, nn:
                                               lambda: qk_chain(m, s, nn))(
                                                   mt, sec, n)))

                    def push_vproj(tts):
                        for tt in tts:
                            projq.append(((tt // 4, 0, 1),
                                          (lambda t: lambda: emit_vproj([t]))(tt)))

                    def push_out(mts):
                        for mt in mts:
                            for n in range(2):
                                outq.append(
                                    (lambda m, nn:
                                     lambda: out_chain(m, nn))(mt, n))

                    for qt in range(NQT):
                        if qt == 0:
                            emit_qkproj(0, ns=[0])
                            push_vproj(range(0, 4))
                            push_qk(0, mts=range(1, NMT))
                            push_vproj(range(4, 8))
                            push_qk(1)
                        elif qt < 3:
                            push_vproj(range(4 * (qt + 1), 4 * (qt + 1) + 4))
                            push_qk(qt + 1)
                        if qt == 2:
                            push_out(range(0, 4))
                        elif qt == 3:
                            push_out(range(4, 12))
                        for hp in range(HLOC // 2):
                            drain_due(qt, hp)
                            tail = hp == 3 and qt == 3
                            emit_unit(hp, qt, fillers,
                                      per_slot=(4 if qt == 0 else
                                                3 if qt == 1 else
                                                2 if qt == 2 else 1),
                                      tail=tail)
                        if qt < 3:
                            emit_attnT(qt)
                    while outq:
                        outq.popleft()()

    _split_multiwaits(nc)
    return nc


_NC_CACHE = {}


def get_nc():
    if "nc" not in _NC_CACHE:
        _NC_CACHE["nc"] = build_nc()
    return _NC_CACHE["nc"]


E4M3 = ml_dtypes.float8_e4m3


def _split8(a):
    hi = a.astype(E4M3)
    lo = (a - hi.astype(np.float32)).astype(E4M3)
    return hi, lo


def _xlay(a):
    """x slice [T, D] f32 -> fp8 [128, g(4) x {hi,lo}(2) x kk(8) x 512]:
    [p, g, s, kk, u] = split_s(a[g*512 + u, kk*128 + p])."""
    hi, lo = _split8(a)

    def lay(t):
        # [T, D] -> [g, 512, kk, 128] -> [128(p), g, kk, 512(u)]
        t4 = t.reshape(NQT, 512, NDIN, 128)
        return t4.transpose(3, 0, 2, 1)

    return np.ascontiguousarray(
        np.stack([lay(hi), lay(lo)], axis=2).reshape(128, -1))


def _wqklay(w):
    """w [D, 1024] (q cols | k cols) f32*32 -> fp8
    [128, mt x {hi,lo} x (sec,kk,c)]."""
    hi, lo = _split8(w)

    def lay(t):
        # [kk, 128(p), sec, mt, 128(c)] -> [p, mt, sec, kk, c]
        t5 = t.reshape(NDIN, 128, 2, NMT, 128)
        return t5.transpose(1, 3, 2, 0, 4).reshape(128, NMT, 2048)

    return np.ascontiguousarray(
        np.stack([lay(hi), lay(lo)], axis=2).reshape(128, -1))


def _wvlay(w):
    """w [D, 512] f32*32 -> fp8 [128, {hi|lo} x kk*512]."""
    hi, lo = _split8(w)

    def lay(t):
        t3 = t.reshape(NDIN, 128, 512)
        return t3.transpose(1, 0, 2).reshape(128, NDIN * 512)

    return np.ascontiguousarray(
        np.concatenate([lay(hi), lay(lo)], axis=1))


def _wolay(w):
    """w [512, 1024] f32*32 -> fp8 [128, {hi|lo} x kk*1024]."""
    hi, lo = _split8(w)

    def lay(t):
        t3 = t.reshape(4, 128, D)
        return t3.transpose(1, 0, 2).reshape(128, 4 * D)

    return np.ascontiguousarray(
        np.concatenate([lay(hi), lay(lo)], axis=1))


def make_in_maps(x, W_qkv, b_qkv, W_out):
    in_maps = []
    for c in range(8):
        b, g = c // 2, c % 2
        s = slice(512 * g, 512 * (g + 1))
        wqk = np.concatenate(
            [W_qkv[:, 512 * g:512 * (g + 1)],
             W_qkv[:, 1024 + 512 * g:1024 + 512 * (g + 1)]],
            axis=1).astype(np.float32) * WSCALE
        wv = W_qkv[:, 2048 + 512 * g:2048 + 512 * (g + 1)].astype(np.float32) * WSCALE
        bslice = np.concatenate(
            [b_qkv[512 * g:512 * (g + 1)],
             b_qkv[1024 + 512 * g:1024 + 512 * (g + 1)],
             b_qkv[2048 + 512 * g:2048 + 512 * (g + 1)]]).astype(np.float32) * WSCALE
        in_maps.append({
            "x8": _xlay(np.asarray(x[b], dtype=np.float32)),
            "wqk8": _wqklay(wqk),
            "wv8": _wvlay(wv),
            "wo8": _wolay(W_out[s].astype(np.float32) * WSCALE),
            "b_qkv": bslice,
        })
    return in_maps


def kernel(x, W_qkv, b_qkv, W_out, b_out):
    x = np.asarray(x)
    W_qkv = np.asarray(W_qkv)
    b_qkv = np.asarray(b_qkv)
    W_out = np.asarray(W_out)
    b_out = np.asarray(b_out)
    nc = get_nc()
    in_maps = make_in_maps(x, W_qkv, b_qkv, W_out)
    res = run_bass_kernel_spmd(nc, in_maps, core_ids=list(range(8))).results
    out = np.stack(
        [res[2 * b]["out"].astype(np.float32)
         + res[2 * b + 1]["out"].astype(np.float32) for b in range(4)], axis=0)
    out = out + b_out[None, None, :].astype(np.float32)
    return out.astype(np.float32)
